# revision 1
# baseline (speedup 1.0000x reference)
"""Trainium2 Bass kernel for nn_CrossTowerCausalModel.

Data-parallel over graphs: each of the 8 NeuronCores handles 128 graphs
(128*32 = 4096 nodes, 128*64 = 8192 edges). Weights/embeddings replicated.

Device activation layout is "transposed" (layout B): hT[feature, node] with
the 768 feature dim split into 6 chunks of 128 partitions. Weight matrices
[in, out] then serve directly as matmul lhsT (stationary) operands.

Host-side prep (pure index logic + layout, no heavy math):
  * per-graph node permutation so that target node c sits at local slot 0 and
    t at slot 1 -> final gathers h_c / h_t become stride-32 strided copies.
    (c == t graphs are fixed up on device with a predicated copy.)
  * x is passed pre-transposed (feature-major) in bf16 so the projection
    needs no on-device transposes.
  * dense per-graph adjacency (A[t,s] = edge multiplicity), laid out as
    block-diagonal 128x128 tiles covering 4 graphs each -> segment_sum
    becomes small dense matmuls.
  * the quirky first-edge/dist logic of the reference (exact int math).
  * speaker/emotion one-hots (16 rows) fused into the input projection.

All matmuls run with bf16 inputs (full PE rate; fp32 would be 4x slower and
float32r is rejected by the BIR verifier unless every producer rounds to it).
PSUM accumulation and the GNN residual chain stay in fp32, so per-layer
rounding does not compound across layers.
"""

import numpy as np
import ml_dtypes

B = 1024          # graphs
P = 32            # nodes per graph
N = B * P
H = 768
HC = H // 128     # 6 feature chunks
L = 3
DSEM = 1024
NUM_SPK, NUM_EMO = 9, 7
NCORES = 8
BC = B // NCORES          # graphs per core = 128
NCN = BC * P              # nodes per core = 4096
NT = 8                    # node tiles of 512 per core
GPT = 4                   # groups (of 128 nodes) per node tile

BF16 = ml_dtypes.bfloat16

_cache = {}


def _build_program():
    from contextlib import ExitStack

    import concourse.bacc as bacc
    import concourse.mybir as mybir
    import concourse.tile as tile
    from concourse.masks import make_identity

    f32 = mybir.dt.float32
    bf16 = mybir.dt.bfloat16
    AF = mybir.ActivationFunctionType

    nc = bacc.Bacc(
        "TRN2", target_bir_lowering=False, debug=False, num_devices=NCORES
    )

    dram = lambda name, shape, dt: nc.dram_tensor(
        name, shape, dt, kind="ExternalInput"
    ).ap()

    xt = dram("xt", [DSEM, NCN], bf16)
    oh16 = dram("oh16", [16, NCN], bf16)
    embcat = dram("embcat", [16, H], bf16)
    wsem = dram("wsem", [DSEM, H], bf16)
    wself = dram("wself", [L, H, H], bf16)
    wnbr = dram("wnbr", [L, H, H], bf16)
    atb = dram("atb", [NCN // 128, 128, 128], bf16)
    cmask = dram("cmask", [128, BC], mybir.dt.uint8)
    ohd = dram("ohd", [P, BC], bf16)
    demb = dram("demb", [P, H], bf16)
    wexpl = dram("wexpl", [H, H], bf16)
    bexpl = dram("bexpl", [128, HC], f32)
    ext = dram("ext", [H, BC], bf16)
    wp1 = dram("wp1", [6 * H, H], bf16)
    bp1 = dram("bp1", [128, HC], f32)
    wp2 = dram("wp2", [128, HC], bf16)
    bp2 = dram("bp2", [1, 1], f32)
    out_ap = nc.dram_tensor("out", [1, BC], f32, kind="ExternalOutput").ap()

    # [C*128, J] dram AP -> [128, C, J] (partition-major chunked view)
    def chunked(ap, J):
        return ap.rearrange("(c p) j -> c p j", p=128).transpose([1, 0, 2])

    # SBUF tile [128, C*J] -> [128, C, J]
    def sb3(t, J):
        return t[:].rearrange("p (c j) -> p c j", j=J)

    with tile.TileContext(nc) as tc, ExitStack() as ctx:
        erpool = ctx.enter_context(tc.tile_pool(name="er", bufs=1))

        hs = ctx.enter_context(ExitStack())
        hpool = hs.enter_context(tc.tile_pool(name="h", bufs=1))
        hconst = hs.enter_context(tc.tile_pool(name="hconst", bufs=1))

        ident = hconst.tile([128, 128], bf16)
        make_identity(nc, ident)
        atb_t = hconst.tile([128, (NCN // 128) * 128], bf16)
        nc.sync.dma_start(sb3(atb_t, 128), atb.transpose([1, 0, 2]))
        cmask_t = hconst.tile([128, BC], mybir.dt.uint8)
        nc.sync.dma_start(cmask_t[:], cmask[:])

        # persistent transposed activations: hT[jc][nt] is [128, 512] fp32
        hT = [
            [
                hpool.tile(
                    [128, 512], f32, tag=f"h_{jc}_{nt}", name=f"h_{jc}_{nt}"
                )
                for nt in range(NT)
            ]
            for jc in range(HC)
        ]
        # edge_repr^T, 36 chunks of 128 rows: [h_graph_c, h_text_c, h_graph_t,
        # h_text_t, h_dist, z] each HC chunks wide
        erT = erpool.tile([128, 36 * 128], bf16)

        def gather_ct(base_c, base_t):
            # strided gathers of node slot 0 (c) and slot 1 (t) per graph,
            # plus the c==t fixup via predicated copy
            for jc in range(HC):
                for nt in range(NT):
                    src = hT[jc][nt].rearrange("p (b u) -> p b u", u=P)
                    nc.vector.tensor_copy(
                        erT[:, (base_c + jc) * 128 + nt * 16:][:, :16],
                        src[:, :, 0],
                    )
                    nc.vector.tensor_copy(
                        erT[:, (base_t + jc) * 128 + nt * 16:][:, :16],
                        src[:, :, 1],
                    )
                nc.vector.copy_predicated(
                    erT[:, (base_t + jc) * 128:][:, :BC],
                    cmask_t[:],
                    erT[:, (base_c + jc) * 128:][:, :BC],
                )

        # ---------------- phase 1: text projection ----------------
        with ExitStack() as p1:
            xtpool = p1.enter_context(tc.tile_pool(name="xt", bufs=3))
            wsem_pool = p1.enter_context(tc.tile_pool(name="wsem", bufs=1))
            oh_pool = p1.enter_context(tc.tile_pool(name="oh16", bufs=3))
            ps_a = p1.enter_context(tc.tile_pool(name="ps_a", bufs=4, space="PSUM"))

            wsem_t = wsem_pool.tile([128, 8 * H], bf16)
            nc.sync.dma_start(sb3(wsem_t, H), chunked(wsem, H))
            emb_t = wsem_pool.tile([128, H], bf16)
            nc.sync.dma_start(emb_t[:16, :], embcat[:])
            for nt in range(NT):
                oh16_t = oh_pool.tile([128, 512], bf16)
                nc.sync.dma_start(oh16_t[:16, :], oh16[:, nt * 512:][:, :512])
                xt_t = xtpool.tile([128, 8 * 512], bf16)
                nc.sync.dma_start(
                    sb3(xt_t, 512), chunked(xt[:, nt * 512:][:, :512], 512)
                )
                for jc in range(HC):
                    acc = ps_a.tile([128, 512], f32)
                    for kc in range(8):
                        nc.tensor.matmul(
                            acc[:],
                            wsem_t[:, kc * H + jc * 128:][:, :128],
                            xt_t[:, kc * 512:][:, :512],
                            start=(kc == 0),
                            stop=False,
                        )
                    nc.tensor.matmul(
                        acc[:],
                        emb_t[:16, jc * 128:][:, :128],
                        oh16_t[:16, :],
                        start=False,
                        stop=True,
                    )
                    nc.scalar.activation(hT[jc][nt][:], acc[:], AF.Relu)

            # h_text gathers (chunks 6-11 = h_text_c, 18-23 = h_text_t)
            gather_ct(6, 18)

        # ---------------- phase 2: GNN layers ----------------
        with ExitStack() as p2:
            wpool = p2.enter_context(tc.tile_pool(name="w", bufs=2))
            hbpool = p2.enter_context(tc.tile_pool(name="hb", bufs=2))
            hapool = p2.enter_context(tc.tile_pool(name="ha", bufs=3))
            msgpool = p2.enter_context(tc.tile_pool(name="msg", bufs=2))
            tmppool = p2.enter_context(tc.tile_pool(name="tmp", bufs=3))
            ps_t2 = p2.enter_context(tc.tile_pool(name="ps_t2", bufs=2, space="PSUM"))
            ps_m = p2.enter_context(tc.tile_pool(name="ps_m", bufs=2, space="PSUM"))
            ps_a2 = p2.enter_context(tc.tile_pool(name="ps_a2", bufs=3, space="PSUM"))

            for l in range(L):
                ws_t = wpool.tile([128, HC * H], bf16, tag="ws")
                nc.sync.dma_start(sb3(ws_t, H), chunked(wself[l], H))
                wn_t = wpool.tile([128, HC * H], bf16, tag="wn")
                nc.sync.dma_start(sb3(wn_t, H), chunked(wnbr[l], H))
                for nt in range(NT):
                    # bf16 copy of this node-tile of hT (matmul/transpose input)
                    hb = hbpool.tile([128, HC * 512], bf16)
                    for jc in range(HC):
                        nc.vector.tensor_copy(
                            hb[:, jc * 512:][:, :512], hT[jc][nt][:]
                        )
                    msg_t = msgpool.tile([128, HC * 512], bf16)
                    for jc in range(HC):
                        # 4 groups' transposes packed into one psum bank
                        pst = ps_t2.tile([128, 512], bf16)
                        for g4 in range(GPT):
                            nc.tensor.transpose(
                                pst[:, g4 * 128:][:, :128],
                                hb[:, jc * 512 + g4 * 128:][:, :128],
                                ident[:],
                            )
                        # ha[s, (g4, j-of-chunk-jc)]
                        ha = hapool.tile([128, 512], bf16)
                        nc.scalar.activation(ha[:], pst[:], AF.Copy)
                        psm = ps_m.tile([128, 512], f32)
                        for g4 in range(GPT):
                            nc.tensor.matmul(
                                psm[:, g4 * 128:][:, :128],
                                ha[:, g4 * 128:][:, :128],
                                atb_t[:, (nt * GPT + g4) * 128:][:, :128],
                                start=True,
                                stop=True,
                            )
                        nc.vector.tensor_copy(msg_t[:, jc * 512:][:, :512], psm[:])
                    for jc in range(HC):
                        acc = ps_a2.tile([128, 512], f32)
                        for kc in range(HC):
                            nc.tensor.matmul(
                                acc[:],
                                ws_t[:, kc * H + jc * 128:][:, :128],
                                hb[:, kc * 512:][:, :512],
                                start=(kc == 0),
                                stop=False,
                            )
                        for kc in range(HC):
                            nc.tensor.matmul(
                                acc[:],
                                wn_t[:, kc * H + jc * 128:][:, :128],
                                msg_t[:, kc * 512:][:, :512],
                                start=False,
                                stop=(kc == HC - 1),
                            )
                        tmp = tmppool.tile([128, 512], f32)
                        nc.scalar.activation(tmp[:], acc[:], AF.Relu)
                        nc.vector.tensor_add(
                            out=hT[jc][nt][:], in0=tmp[:], in1=hT[jc][nt][:]
                        )

        # final h gathers (chunks 0-5 = h_graph_c, 12-17 = h_graph_t)
        gather_ct(0, 12)
        # release hT + GNN constants before the predictor phase
        hs.close()

        # ---------------- phase 3: predictor ----------------
        with ExitStack() as p3:
            ppool = p3.enter_context(tc.tile_pool(name="pred", bufs=1))
            ps_p = p3.enter_context(tc.tile_pool(name="ps_p", bufs=2, space="PSUM"))

            wp1_t = []
            for jc in range(HC):
                w1s = ppool.tile(
                    [128, 36 * 128], bf16, tag=f"wp1_{jc}", name=f"wp1_{jc}"
                )
                nc.sync.dma_start(
                    sb3(w1s, 128), chunked(wp1[:, jc * 128:][:, :128], 128)
                )
                wp1_t.append(w1s)
            ohd_t = ppool.tile([128, BC], bf16)
            nc.sync.dma_start(ohd_t[:P, :], ohd[:])
            demb_t = ppool.tile([128, H], bf16)
            nc.sync.dma_start(demb_t[:P, :], demb[:])
            bexpl_t = ppool.tile([128, HC], f32)
            nc.sync.dma_start(bexpl_t[:], bexpl[:])
            bp1_t = ppool.tile([128, HC], f32)
            nc.sync.dma_start(bp1_t[:], bp1[:])
            wp2_t = ppool.tile([128, HC], bf16)
            nc.sync.dma_start(wp2_t[:], wp2[:])
            bp2_t = ppool.tile([1, 1], f32)
            nc.sync.dma_start(bp2_t[:], bp2[:])
            ext_t = ppool.tile([128, HC * BC], bf16)
            nc.sync.dma_start(sb3(ext_t, BC), chunked(ext, BC))
            wexpl_t = ppool.tile([128, HC * H], bf16)
            nc.sync.dma_start(sb3(wexpl_t, H), chunked(wexpl, H))

            # h_dist (chunks 24-29)
            for jc in range(HC):
                psd = ps_p.tile([128, BC], f32)
                nc.tensor.matmul(
                    psd[:],
                    demb_t[:P, jc * 128:][:, :128],
                    ohd_t[:P, :],
                    start=True,
                    stop=True,
                )
                nc.scalar.activation(erT[:, (24 + jc) * 128:][:, :BC], psd[:], AF.Copy)

            # z_teacher (chunks 30-35)
            for jc in range(HC):
                psz = ps_p.tile([128, BC], f32)
                for kc in range(HC):
                    nc.tensor.matmul(
                        psz[:],
                        wexpl_t[:, kc * H + jc * 128:][:, :128],
                        ext_t[:, kc * BC:][:, :BC],
                        start=(kc == 0),
                        stop=(kc == HC - 1),
                    )
                nc.scalar.activation(
                    erT[:, (30 + jc) * 128:][:, :BC],
                    psz[:],
                    AF.Relu,
                    bias=bexpl_t[:, jc:jc + 1],
                )

            hid_t = ppool.tile([128, HC * BC], bf16)
            for jc in range(HC):
                psp = ps_p.tile([128, BC], f32)
                for kc in range(36):
                    nc.tensor.matmul(
                        psp[:],
                        wp1_t[jc][:, kc * 128:][:, :128],
                        erT[:, kc * 128:][:, :128],
                        start=(kc == 0),
                        stop=(kc == 35),
                    )
                nc.scalar.activation(
                    hid_t[:, jc * BC:][:, :BC],
                    psp[:],
                    AF.Relu,
                    bias=bp1_t[:, jc:jc + 1],
                )

            psl = ps_p.tile([128, BC], f32)
            for jc in range(HC):
                nc.tensor.matmul(
                    psl[:1, :],
                    wp2_t[:, jc:jc + 1],
                    hid_t[:, jc * BC:][:, :BC],
                    start=(jc == 0),
                    stop=(jc == HC - 1),
                )
            logit_t = ppool.tile([128, BC], f32)
            nc.vector.tensor_scalar_add(
                out=logit_t[:1, :], in0=psl[:1, :], scalar1=bp2_t[:1, :1]
            )
            nc.sync.dma_start(out_ap[:], logit_t[:1, :])

    nc.compile()
    return nc


def _host_prep(inputs):
    x = np.asarray(inputs["x"], np.float32)
    spk = np.asarray(inputs["speaker_ids"], np.int64)
    emo = np.asarray(inputs["emotion_ids"], np.int64)
    ei = np.asarray(inputs["edge_index"], np.int64)
    tni = np.asarray(inputs["target_node_indices"], np.int64)
    ex = np.asarray(inputs["expl_space_vec"], np.float32)

    E = ei.shape[1]
    edge_src, edge_tgt = ei[0], ei[1]
    c_idx, t_idx = tni[:, 0], tni[:, 1]

    # reference first-edge/dist logic (exact)
    fe = np.full(N, E, np.int64)
    np.minimum.at(fe, edge_src, np.arange(E, dtype=np.int64))

    def first_tgt(q):
        feq = fe[q]
        return np.where(feq < E, edge_tgt[np.minimum(feq, E - 1)], q)

    dist = np.clip(np.abs(first_tgt(c_idx) - first_tgt(t_idx)), 0, P - 1)

    # per-graph permutation: slot 0 = c, slot 1 = t (if distinct)
    prio = np.full((B, P), 2, np.int64)
    prio[np.arange(B), t_idx] = 1
    prio[np.arange(B), c_idx] = 0
    new2old = np.argsort(prio, axis=1, kind="stable")
    old2new = np.argsort(new2old, axis=1)
    perm_global = (np.arange(B)[:, None] * P + new2old).reshape(-1)

    xtb = np.ascontiguousarray(x[perm_global].T.astype(BF16))  # [DSEM, N]
    spk_new = spk[perm_global]
    emo_new = emo[perm_global]

    oh16 = np.zeros((16, N), BF16)
    oh16[spk_new, np.arange(N)] = 1.0
    oh16[NUM_SPK + emo_new, np.arange(N)] = 1.0

    # adjacency in permuted coords, block-diag AT tiles (4 graphs/tile)
    g_e = edge_src // P
    s_new = old2new[g_e, edge_src % P]
    t_new = old2new[g_e, edge_tgt % P]
    A = np.zeros((B, P, P), np.float32)
    np.add.at(A, (g_e, t_new, s_new), 1.0)
    G = B // 4
    atb = np.zeros((G, 128, 128), np.float32)
    Ar = A.reshape(G, 4, P, P)
    for i in range(4):
        atb[:, 32 * i:32 * i + 32, 32 * i:32 * i + 32] = Ar[:, i].transpose(0, 2, 1)
    atb = atb.astype(BF16)

    cmask = np.tile((c_idx == t_idx).astype(np.uint8)[None, :], (128, 1))

    ohd = np.zeros((P, B), BF16)
    ohd[dist, np.arange(B)] = 1.0

    extT = np.ascontiguousarray(ex.T.astype(BF16))

    embcat = np.concatenate(
        [np.asarray(inputs["spk_emb"], np.float32),
         np.asarray(inputs["emo_emb"], np.float32)], 0
    ).astype(BF16)
    rearr = lambda v: np.ascontiguousarray(
        np.asarray(v, np.float32).reshape(HC, 128).T
    )
    b16 = lambda k: np.asarray(inputs[k], np.float32).astype(BF16)

    shared = dict(
        embcat=embcat,
        wsem=b16("W_sem"),
        wself=b16("gnn_w_self"),
        wnbr=b16("gnn_w_nbr"),
        demb=b16("dist_emb"),
        wexpl=b16("W_expl"),
        bexpl=rearr(inputs["b_expl"]),
        wp1=b16("W_p1"),
        bp1=rearr(inputs["b_p1"]),
        wp2=rearr(np.asarray(inputs["W_p2"], np.float32)[:, 0]).astype(BF16),
        bp2=np.asarray(inputs["b_p2"], np.float32).reshape(1, 1),
    )

    in_maps = []
    for i in range(NCORES):
        gs = slice(i * BC, (i + 1) * BC)
        ns = slice(i * NCN, (i + 1) * NCN)
        m = dict(shared)
        m["xt"] = np.ascontiguousarray(xtb[:, ns])
        m["oh16"] = np.ascontiguousarray(oh16[:, ns])
        m["atb"] = np.ascontiguousarray(atb[i * (NCN // 128):(i + 1) * (NCN // 128)])
        m["cmask"] = np.ascontiguousarray(cmask[:, gs])
        m["ohd"] = np.ascontiguousarray(ohd[:, gs])
        m["ext"] = np.ascontiguousarray(extT[:, gs])
        in_maps.append(m)
    return in_maps


def kernel(**inputs):
    in_maps = _host_prep(inputs)
    if "nc" not in _cache:
        _cache["nc"] = _build_program()
    from concourse.bass_utils import run_bass_kernel_spmd

    res = run_bass_kernel_spmd(_cache["nc"], in_maps, list(range(NCORES)))
    out = np.concatenate(
        [res.results[i]["out"].reshape(BC) for i in range(NCORES)]
    )
    return out.astype(np.float32)



# revision 7
# speedup vs baseline: 1.4317x; 1.4317x over previous
"""Trainium2 Bass kernel for nn_CrossTowerCausalModel.

Data-parallel over graphs: each of the 8 NeuronCores handles 128 graphs
(128*32 = 4096 nodes, 128*64 = 8192 edges). Weights/embeddings replicated.

Device activation layout is "transposed" (layout B): hT[feature, node] with
the 768 feature dim split into 6 chunks of 128 partitions. Weight matrices
[in, out] then serve directly as matmul lhsT (stationary) operands.

Receptive-field restriction: the GNN output h_graph is only read at 2 nodes
per graph (c, t). Host permutes each graph's 32 node slots so that
  slot 0 = c, slot 1 = t (filler if c == t),
  slots [0, C2) contain S2 = {c,t} U in({c,t})  (in-neighbors),
so layer 3 only computes slots {0,1} and layer 2 only the C2-slot prefix
(C2=16 when the per-graph |S2| max allows; else full 32). Layer 1 must stay
full (its targets feed layer-2 message sources anywhere). This is exact --
every value read downstream is identical to the full computation.

h is stored in bf16 (matmul input dtype), eliminating the per-layer f32->
bf16 copy; the per-layer psum accumulation and relu stay fp32.

Host-side prep (pure index logic + layout, no heavy math):
  * per-graph node permutation (above) -> final gathers h_c / h_t become
    stride-32 strided copies. (c == t graphs fixed up with copy_predicated.)
  * x passed pre-transposed (feature-major) bf16.
  * dense per-graph adjacency as block-diagonal 128x128 AT tiles (layer 1),
    [128, 4*C2] restricted tiles (layer 2), [128, 16] tiles (layer 3).
  * the quirky first-edge/dist logic of the reference (exact int math).
  * speaker/emotion one-hots (16 rows) fused into the input projection.
"""

import numpy as np
import ml_dtypes

B = 1024          # graphs
P = 32            # nodes per graph
N = B * P
H = 768
HC = H // 128     # 6 feature chunks
L = 3
DSEM = 1024
NUM_SPK, NUM_EMO = 9, 7
NCORES = 8
BC = B // NCORES          # graphs per core = 128
NCN = BC * P              # nodes per core = 4096
NT = 8                    # node tiles of 512 per core
GPT = 4                   # groups (of 128 nodes) per node tile

BF16 = ml_dtypes.bfloat16

_cache = {}


def _build_program(C2):
    from contextlib import ExitStack

    import concourse.bacc as bacc
    import concourse.mybir as mybir
    import concourse.tile as tile
    from concourse.masks import make_identity

    f32 = mybir.dt.float32
    bf16 = mybir.dt.bfloat16
    AF = mybir.ActivationFunctionType

    NC2 = BC * C2             # layer-2 packed cols per core
    NT2 = NC2 // 512          # layer-2 node tiles of 512
    RT3 = NC2 // 128          # layer-3 source row tiles
    GP3 = 128 // C2           # graphs per layer-3 source row tile

    nc = bacc.Bacc(
        "TRN2", target_bir_lowering=False, debug=False, num_devices=NCORES
    )

    dram = lambda name, shape, dt: nc.dram_tensor(
        name, shape, dt, kind="ExternalInput"
    ).ap()

    xt = dram("xt", [DSEM, NCN], bf16)
    oh16 = dram("oh16", [16, NCN], bf16)
    embcat = dram("embcat", [16, H], bf16)
    wsem = dram("wsem", [DSEM, H], bf16)
    wself = dram("wself", [L, H, H], bf16)
    wnbr = dram("wnbr", [L, H, H], bf16)
    atb = dram("atb", [NCN // 128, 128, 128], bf16)
    atb2 = dram("atb2", [NCN // 128, 128, 4 * C2], bf16)
    atb3 = dram("atb3", [RT3, 128, 2 * GP3], bf16)
    cmask = dram("cmask", [128, BC], mybir.dt.uint8)
    ohd = dram("ohd", [P, BC], bf16)
    demb = dram("demb", [P, H], bf16)
    wexpl = dram("wexpl", [H, H], bf16)
    bexpl = dram("bexpl", [128, HC], f32)
    ext = dram("ext", [H, BC], bf16)
    wp1 = dram("wp1", [6 * H, H], bf16)
    bp1 = dram("bp1", [128, HC], f32)
    wp2 = dram("wp2", [128, HC], bf16)
    bp2 = dram("bp2", [1, 1], f32)
    out_ap = nc.dram_tensor("out", [1, BC], f32, kind="ExternalOutput").ap()

    # [C*128, J] dram AP -> [128, C, J] (partition-major chunked view)
    def chunked(ap, J):
        return ap.rearrange("(c p) j -> c p j", p=128).transpose([1, 0, 2])

    # SBUF tile [128, C*J] -> [128, C, J]
    def sb3(t, J):
        return t[:].rearrange("p (c j) -> p c j", j=J)

    with tile.TileContext(nc) as tc, ExitStack() as ctx:
        erpool = ctx.enter_context(tc.tile_pool(name="er", bufs=1))
        cpool = ctx.enter_context(tc.tile_pool(name="const", bufs=1))

        # pools close in LIFO order: hA (after layer 1), then hB (after
        # layer 2); h2 lives until the end.
        h2pool = ctx.enter_context(tc.tile_pool(name="h2", bufs=1))
        hsB = ctx.enter_context(ExitStack())     # closes after layer 2
        hBpool = hsB.enter_context(tc.tile_pool(name="hB", bufs=1))
        hsA = ctx.enter_context(ExitStack())     # closes after layer 1
        hApool = hsA.enter_context(tc.tile_pool(name="hA", bufs=1))

        ident = cpool.tile([128, 128], bf16)
        make_identity(nc, ident)
        cmask_t = cpool.tile([128, BC], mybir.dt.uint8)
        nc.sync.dma_start(cmask_t[:], cmask[:])

        # transposed activations, ping-pong: hA = h_text (layer-1 input),
        # hB = h1 (layer-2 input); each [jc][nt] tile is [128, 512] bf16
        hA = [
            [
                hApool.tile(
                    [128, 512], bf16, tag=f"hA_{jc}_{nt}", name=f"hA_{jc}_{nt}"
                )
                for nt in range(NT)
            ]
            for jc in range(HC)
        ]
        hB = [
            [
                hBpool.tile(
                    [128, 512], bf16, tag=f"hB_{jc}_{nt}", name=f"hB_{jc}_{nt}"
                )
                for nt in range(NT)
            ]
            for jc in range(HC)
        ]
        # layer-2 packed activations: h2T[jc][nt2] is [128, 512] bf16
        h2T = [
            [
                h2pool.tile(
                    [128, 512], bf16, tag=f"h2_{jc}_{nt2}", name=f"h2_{jc}_{nt2}"
                )
                for nt2 in range(NT2)
            ]
            for jc in range(HC)
        ]
        # edge_repr^T, 36 chunks of 128 rows: [h_graph_c, h_text_c, h_graph_t,
        # h_text_t, h_dist, z] each HC chunks wide
        erT = erpool.tile([128, 36 * 128], bf16)

        # ---------------- phase 1: text projection ----------------
        with ExitStack() as p1:
            xtpool = p1.enter_context(tc.tile_pool(name="xt", bufs=3))
            wsem_pool = p1.enter_context(tc.tile_pool(name="wsem", bufs=1))
            oh_pool = p1.enter_context(tc.tile_pool(name="oh16", bufs=3))
            ps_a = p1.enter_context(tc.tile_pool(name="ps_a", bufs=4, space="PSUM"))

            wsem_t = wsem_pool.tile([128, 8 * H], bf16)
            nc.sync.dma_start(sb3(wsem_t, H), chunked(wsem, H))
            emb_t = wsem_pool.tile([128, H], bf16)
            nc.sync.dma_start(emb_t[:16, :], embcat[:])
            for nt in range(NT):
                oh16_t = oh_pool.tile([128, 512], bf16)
                nc.sync.dma_start(oh16_t[:16, :], oh16[:, nt * 512:][:, :512])
                xt_t = xtpool.tile([128, 8 * 512], bf16)
                nc.sync.dma_start(
                    sb3(xt_t, 512), chunked(xt[:, nt * 512:][:, :512], 512)
                )
                for jc in range(HC):
                    acc = ps_a.tile([128, 512], f32)
                    for kc in range(8):
                        nc.tensor.matmul(
                            acc[:],
                            wsem_t[:, kc * H + jc * 128:][:, :128],
                            xt_t[:, kc * 512:][:, :512],
                            start=(kc == 0),
                            stop=False,
                        )
                    nc.tensor.matmul(
                        acc[:],
                        emb_t[:16, jc * 128:][:, :128],
                        oh16_t[:16, :],
                        start=False,
                        stop=True,
                    )
                    nc.scalar.activation(hA[jc][nt][:], acc[:], AF.Relu)

            # h_text gathers (chunks 6-11 = h_text_c, 18-23 = h_text_t)
            for jc in range(HC):
                for nt in range(NT):
                    src = hA[jc][nt].rearrange("p (b u) -> p b u", u=P)
                    nc.vector.tensor_copy(
                        erT[:, (6 + jc) * 128 + nt * 16:][:, :16],
                        src[:, :, 0],
                    )
                    nc.vector.tensor_copy(
                        erT[:, (18 + jc) * 128 + nt * 16:][:, :16],
                        src[:, :, 1],
                    )
                nc.vector.copy_predicated(
                    erT[:, (18 + jc) * 128:][:, :BC],
                    cmask_t[:],
                    erT[:, (6 + jc) * 128:][:, :BC],
                )

        # ---------------- phase 2: GNN layer 1 (full) ----------------
        with ExitStack() as p2:
            wpool = p2.enter_context(tc.tile_pool(name="w", bufs=2))
            a1pool = p2.enter_context(tc.tile_pool(name="a1", bufs=1))
            hapool = p2.enter_context(tc.tile_pool(name="ha", bufs=3))
            msgpool = p2.enter_context(tc.tile_pool(name="msg", bufs=2))
            tmppool = p2.enter_context(tc.tile_pool(name="tmp", bufs=3))
            ps_t2 = p2.enter_context(tc.tile_pool(name="ps_t2", bufs=2, space="PSUM"))
            ps_m = p2.enter_context(tc.tile_pool(name="ps_m", bufs=2, space="PSUM"))
            ps_a2 = p2.enter_context(tc.tile_pool(name="ps_a2", bufs=3, space="PSUM"))

            atb_t = a1pool.tile([128, (NCN // 128) * 128], bf16)
            nc.sync.dma_start(sb3(atb_t, 128), atb.transpose([1, 0, 2]))

            ws_t = wpool.tile([128, HC * H], bf16, tag="ws")
            nc.sync.dma_start(sb3(ws_t, H), chunked(wself[0], H))
            wn_t = wpool.tile([128, HC * H], bf16, tag="wn")
            nc.sync.dma_start(sb3(wn_t, H), chunked(wnbr[0], H))
            for nt in range(NT):
                msg_t = msgpool.tile([128, HC * 512], bf16)
                for jc in range(HC):
                    pst = ps_t2.tile([128, 512], bf16)
                    for g4 in range(GPT):
                        nc.tensor.transpose(
                            pst[:, g4 * 128:][:, :128],
                            hA[jc][nt][:, g4 * 128:][:, :128],
                            ident[:],
                        )
                    ha = hapool.tile([128, 512], bf16)
                    nc.scalar.activation(ha[:], pst[:], AF.Copy)
                    psm = ps_m.tile([128, 512], f32)
                    for g4 in range(GPT):
                        nc.tensor.matmul(
                            psm[:, g4 * 128:][:, :128],
                            ha[:, g4 * 128:][:, :128],
                            atb_t[:, (nt * GPT + g4) * 128:][:, :128],
                            start=True,
                            stop=True,
                        )
                    nc.vector.tensor_copy(msg_t[:, jc * 512:][:, :512], psm[:])
                for jc in range(HC):
                    acc = ps_a2.tile([128, 512], f32)
                    for kc in range(HC):
                        nc.tensor.matmul(
                            acc[:],
                            ws_t[:, kc * H + jc * 128:][:, :128],
                            hA[kc][nt][:],
                            start=(kc == 0),
                            stop=False,
                        )
                    for kc in range(HC):
                        nc.tensor.matmul(
                            acc[:],
                            wn_t[:, kc * H + jc * 128:][:, :128],
                            msg_t[:, kc * 512:][:, :512],
                            start=False,
                            stop=(kc == HC - 1),
                        )
                    tmp = tmppool.tile([128, 512], f32)
                    nc.scalar.activation(tmp[:], acc[:], AF.Relu)
                    nc.vector.tensor_add(
                        out=hB[jc][nt][:], in0=tmp[:], in1=hA[jc][nt][:]
                    )

        # hA (h_text) no longer needed
        hsA.close()

        # ---------------- phase 3: GNN layer 2 (prefix C2) ----------------
        with ExitStack() as p3:
            wpool = p3.enter_context(tc.tile_pool(name="w2", bufs=2))
            a2pool = p3.enter_context(tc.tile_pool(name="a2", bufs=1))
            hapool = p3.enter_context(tc.tile_pool(name="ha2", bufs=3))
            hppool = p3.enter_context(tc.tile_pool(name="hp", bufs=2))
            msgpool = p3.enter_context(tc.tile_pool(name="msg2", bufs=2))
            tmppool = p3.enter_context(tc.tile_pool(name="tmp2", bufs=3))
            ps_t2 = p3.enter_context(tc.tile_pool(name="ps_t3", bufs=2, space="PSUM"))
            ps_m = p3.enter_context(tc.tile_pool(name="ps_m3", bufs=2, space="PSUM"))
            ps_a2 = p3.enter_context(tc.tile_pool(name="ps_a3", bufs=3, space="PSUM"))

            atb2_t = a2pool.tile([128, (NCN // 128) * 4 * C2], bf16)
            nc.sync.dma_start(sb3(atb2_t, 4 * C2), atb2.transpose([1, 0, 2]))

            ws_t = wpool.tile([128, HC * H], bf16, tag="ws2")
            nc.sync.dma_start(sb3(ws_t, H), chunked(wself[1], H))
            wn_t = wpool.tile([128, HC * H], bf16, tag="wn2")
            nc.sync.dma_start(sb3(wn_t, H), chunked(wnbr[1], H))
            W2 = 4 * C2  # target cols produced per source row-tile
            SP2 = 512 // (16 * C2)  # source node-tiles feeding one nt2 tile
            for nt2 in range(NT2):
                # message phase over the two full-layout source tiles
                msg_t = msgpool.tile([128, HC * 512], bf16)
                for jc in range(HC):
                    psm = ps_m.tile([128, 512], f32)
                    for half in range(SP2):
                        nt = nt2 * SP2 + half
                        pst = ps_t2.tile([128, 512], bf16)
                        for g4 in range(GPT):
                            nc.tensor.transpose(
                                pst[:, g4 * 128:][:, :128],
                                hB[jc][nt][:, g4 * 128:][:, :128],
                                ident[:],
                            )
                        ha = hapool.tile([128, 512], bf16)
                        nc.scalar.activation(ha[:], pst[:], AF.Copy)
                        for g4 in range(GPT):
                            nc.tensor.matmul(
                                psm[:, (half * GPT + g4) * W2:][:, :W2],
                                ha[:, g4 * 128:][:, :128],
                                atb2_t[:, (nt * GPT + g4) * W2:][:, :W2],
                                start=True,
                                stop=True,
                            )
                    nc.vector.tensor_copy(msg_t[:, jc * 512:][:, :512], psm[:])
                # gather h1 at the C2-prefix of each graph (residual + self rhs)
                hp_t = hppool.tile([128, HC * 512], bf16)
                for jc in range(HC):
                    for half in range(SP2):
                        nt = nt2 * SP2 + half
                        src = hB[jc][nt].rearrange("p (b u) -> p b u", u=P)
                        dst = hp_t[
                            :, jc * 512 + half * (512 // SP2):
                        ][:, :512 // SP2].rearrange("p (b u) -> p b u", u=C2)
                        nc.vector.tensor_copy(dst, src[:, :, :C2])
                for jc in range(HC):
                    acc = ps_a2.tile([128, 512], f32)
                    for kc in range(HC):
                        nc.tensor.matmul(
                            acc[:],
                            ws_t[:, kc * H + jc * 128:][:, :128],
                            hp_t[:, kc * 512:][:, :512],
                            start=(kc == 0),
                            stop=False,
                        )
                    for kc in range(HC):
                        nc.tensor.matmul(
                            acc[:],
                            wn_t[:, kc * H + jc * 128:][:, :128],
                            msg_t[:, kc * 512:][:, :512],
                            start=False,
                            stop=(kc == HC - 1),
                        )
                    tmp = tmppool.tile([128, 512], f32)
                    nc.scalar.activation(tmp[:], acc[:], AF.Relu)
                    nc.vector.tensor_add(
                        out=h2T[jc][nt2][:],
                        in0=tmp[:],
                        in1=hp_t[:, jc * 512:][:, :512],
                    )

        # hB (h1) no longer needed
        hsB.close()

        # ---------------- phase 4: GNN layer 3 (slots 0,1) + predictor ----
        with ExitStack() as p4:
            ppool = p4.enter_context(tc.tile_pool(name="pred", bufs=1))
            w3pool = p4.enter_context(tc.tile_pool(name="w3", bufs=1))
            a3pool = p4.enter_context(tc.tile_pool(name="a3", bufs=1))
            hapool = p4.enter_context(tc.tile_pool(name="ha3", bufs=3))
            tmppool = p4.enter_context(tc.tile_pool(name="tmp3", bufs=2))
            ps_t2 = p4.enter_context(tc.tile_pool(name="ps_t4", bufs=2, space="PSUM"))
            ps_m = p4.enter_context(tc.tile_pool(name="ps_m4", bufs=1, space="PSUM"))
            ps_c3 = p4.enter_context(tc.tile_pool(name="ps_c3", bufs=2, space="PSUM"))
            ps_p = p4.enter_context(tc.tile_pool(name="ps_p", bufs=3, space="PSUM"))

            # predictor weight prefetch (DMA overlaps layer-3 compute)
            wp1_t = []
            for jc in range(HC):
                w1s = ppool.tile(
                    [128, 36 * 128], bf16, tag=f"wp1_{jc}", name=f"wp1_{jc}"
                )
                nc.sync.dma_start(
                    sb3(w1s, 128), chunked(wp1[:, jc * 128:][:, :128], 128)
                )
                wp1_t.append(w1s)
            ohd_t = ppool.tile([128, BC], bf16)
            nc.sync.dma_start(ohd_t[:P, :], ohd[:])
            demb_t = ppool.tile([128, H], bf16)
            nc.sync.dma_start(demb_t[:P, :], demb[:])
            bexpl_t = ppool.tile([128, HC], f32)
            nc.sync.dma_start(bexpl_t[:], bexpl[:])
            bp1_t = ppool.tile([128, HC], f32)
            nc.sync.dma_start(bp1_t[:], bp1[:])
            wp2_t = ppool.tile([128, HC], bf16)
            nc.sync.dma_start(wp2_t[:], wp2[:])
            bp2_t = ppool.tile([1, 1], f32)
            nc.sync.dma_start(bp2_t[:], bp2[:])
            ext_t = ppool.tile([128, HC * BC], bf16)
            nc.sync.dma_start(sb3(ext_t, BC), chunked(ext, BC))
            wexpl_t = ppool.tile([128, HC * H], bf16)
            nc.sync.dma_start(sb3(wexpl_t, H), chunked(wexpl, H))
            atb3_t = a3pool.tile([128, RT3 * 2 * GP3], bf16)
            nc.sync.dma_start(sb3(atb3_t, 2 * GP3), atb3.transpose([1, 0, 2]))
            ws3_t = w3pool.tile([128, HC * H], bf16, tag="ws3")
            nc.sync.dma_start(sb3(ws3_t, H), chunked(wself[2], H))
            wn3_t = w3pool.tile([128, HC * H], bf16, tag="wn3")
            nc.sync.dma_start(sb3(wn3_t, H), chunked(wnbr[2], H))

            # --- layer 3 message + gather at slots {0,1} ---
            W3 = 2 * GP3  # target cols per source row tile
            msg3_t = ppool.tile([128, HC * 2 * BC], bf16)
            h2p_t = ppool.tile([128, HC * 2 * BC], bf16)
            for jc in range(HC):
                psm = ps_m.tile([128, 2 * BC], f32)
                for nt2 in range(NT2):
                    pst = ps_t2.tile([128, 512], bf16)
                    for g4 in range(GPT):
                        nc.tensor.transpose(
                            pst[:, g4 * 128:][:, :128],
                            h2T[jc][nt2][:, g4 * 128:][:, :128],
                            ident[:],
                        )
                    ha = hapool.tile([128, 512], bf16)
                    nc.scalar.activation(ha[:], pst[:], AF.Copy)
                    for g4 in range(GPT):
                        rt = nt2 * GPT + g4
                        nc.tensor.matmul(
                            psm[:, rt * W3:][:, :W3],
                            ha[:, g4 * 128:][:, :128],
                            atb3_t[:, rt * W3:][:, :W3],
                            start=True,
                            stop=True,
                        )
                nc.vector.tensor_copy(msg3_t[:, jc * 2 * BC:][:, :2 * BC], psm[:])
                PW3 = 2 * BC // NT2
                for nt2 in range(NT2):
                    src = h2T[jc][nt2].rearrange("p (b u) -> p b u", u=C2)
                    dst = h2p_t[
                        :, jc * 2 * BC + nt2 * PW3:
                    ][:, :PW3].rearrange("p (b u) -> p b u", u=2)
                    nc.vector.tensor_copy(dst, src[:, :, :2])

            # --- layer 3 W-matmuls -> h3 -> erT chunks 0-5 (c), 12-17 (t) ---
            for jc in range(HC):
                acc = ps_c3.tile([128, 2 * BC], f32)
                for kc in range(HC):
                    nc.tensor.matmul(
                        acc[:],
                        ws3_t[:, kc * H + jc * 128:][:, :128],
                        h2p_t[:, kc * 2 * BC:][:, :2 * BC],
                        start=(kc == 0),
                        stop=False,
                    )
                for kc in range(HC):
                    nc.tensor.matmul(
                        acc[:],
                        wn3_t[:, kc * H + jc * 128:][:, :128],
                        msg3_t[:, kc * 2 * BC:][:, :2 * BC],
                        start=False,
                        stop=(kc == HC - 1),
                    )
                tmp = tmppool.tile([128, 2 * BC], f32)
                nc.scalar.activation(tmp[:], acc[:], AF.Relu)
                h3 = tmppool.tile([128, 2 * BC], bf16)
                nc.vector.tensor_add(
                    out=h3[:], in0=tmp[:], in1=h2p_t[:, jc * 2 * BC:][:, :2 * BC]
                )
                h3v = h3.rearrange("p (b u) -> p b u", u=2)
                nc.vector.tensor_copy(erT[:, (0 + jc) * 128:][:, :BC], h3v[:, :, 0])
                nc.vector.tensor_copy(erT[:, (12 + jc) * 128:][:, :BC], h3v[:, :, 1])
                nc.vector.copy_predicated(
                    erT[:, (12 + jc) * 128:][:, :BC],
                    cmask_t[:],
                    erT[:, (0 + jc) * 128:][:, :BC],
                )

            # h_dist (chunks 24-29)
            for jc in range(HC):
                psd = ps_p.tile([128, BC], f32, tag="pp")
                nc.tensor.matmul(
                    psd[:],
                    demb_t[:P, jc * 128:][:, :128],
                    ohd_t[:P, :],
                    start=True,
                    stop=True,
                )
                nc.scalar.activation(erT[:, (24 + jc) * 128:][:, :BC], psd[:], AF.Copy)

            # z_teacher (chunks 30-35)
            for jc in range(HC):
                psz = ps_p.tile([128, BC], f32, tag="pp")
                for kc in range(HC):
                    nc.tensor.matmul(
                        psz[:],
                        wexpl_t[:, kc * H + jc * 128:][:, :128],
                        ext_t[:, kc * BC:][:, :BC],
                        start=(kc == 0),
                        stop=(kc == HC - 1),
                    )
                nc.scalar.activation(
                    erT[:, (30 + jc) * 128:][:, :BC],
                    psz[:],
                    AF.Relu,
                    bias=bexpl_t[:, jc:jc + 1],
                )

            hid_t = ppool.tile([128, HC * BC], bf16)
            for jc in range(HC):
                psp = ps_p.tile([128, BC], f32, tag="pp")
                for kc in range(36):
                    nc.tensor.matmul(
                        psp[:],
                        wp1_t[jc][:, kc * 128:][:, :128],
                        erT[:, kc * 128:][:, :128],
                        start=(kc == 0),
                        stop=(kc == 35),
                    )
                nc.scalar.activation(
                    hid_t[:, jc * BC:][:, :BC],
                    psp[:],
                    AF.Relu,
                    bias=bp1_t[:, jc:jc + 1],
                )

            psl = ps_p.tile([128, BC], f32, tag="pp")
            for jc in range(HC):
                nc.tensor.matmul(
                    psl[:1, :],
                    wp2_t[:, jc:jc + 1],
                    hid_t[:, jc * BC:][:, :BC],
                    start=(jc == 0),
                    stop=(jc == HC - 1),
                )
            logit_t = ppool.tile([128, BC], f32)
            nc.vector.tensor_scalar_add(
                out=logit_t[:1, :], in0=psl[:1, :], scalar1=bp2_t[:1, :1]
            )
            nc.sync.dma_start(out_ap[:], logit_t[:1, :])

    nc.compile()
    return nc


def _host_prep(inputs):
    x = np.asarray(inputs["x"], np.float32)
    spk = np.asarray(inputs["speaker_ids"], np.int64)
    emo = np.asarray(inputs["emotion_ids"], np.int64)
    ei = np.asarray(inputs["edge_index"], np.int64)
    tni = np.asarray(inputs["target_node_indices"], np.int64)
    ex = np.asarray(inputs["expl_space_vec"], np.float32)

    E = ei.shape[1]
    edge_src, edge_tgt = ei[0], ei[1]
    c_idx, t_idx = tni[:, 0], tni[:, 1]

    # reference first-edge/dist logic (exact)
    fe = np.full(N, E, np.int64)
    np.minimum.at(fe, edge_src, np.arange(E, dtype=np.int64))

    def first_tgt(q):
        feq = fe[q]
        return np.where(feq < E, edge_tgt[np.minimum(feq, E - 1)], q)

    dist = np.clip(np.abs(first_tgt(c_idx) - first_tgt(t_idx)), 0, P - 1)

    # slot-1 node: t, or a filler distinct from c when c == t
    t_eff = np.where(c_idx == t_idx, (t_idx + 1) % P, t_idx)

    # per-graph in-neighbor sets of {c, t_eff} -> S2 (old coords)
    g_e = edge_src // P
    s_l, t_l = edge_src % P, edge_tgt % P
    innb = np.zeros((B, P, P), bool)
    innb[g_e, t_l, s_l] = True
    sel = np.zeros((B, P), bool)
    bidx = np.arange(B)
    sel[bidx, c_idx] = True
    sel[bidx, t_eff] = True
    S2 = sel.copy()
    S2 |= np.einsum("bts,bt->bs", innb.astype(np.int8), sel.astype(np.int8)) > 0
    s2_max = int(S2.sum(1).max())
    C2 = 16 if s2_max <= 16 else 32

    # per-graph permutation: slot 0 = c, slot 1 = t_eff, S2 within prefix C2
    prio = np.full((B, P), 4, np.int64)
    prio[S2] = 2
    prio[bidx, t_eff] = 1
    prio[bidx, c_idx] = 0
    new2old = np.argsort(prio, axis=1, kind="stable")
    old2new = np.argsort(new2old, axis=1)
    perm_global = (np.arange(B)[:, None] * P + new2old).reshape(-1)

    xtb = np.ascontiguousarray(x[perm_global].T.astype(BF16))  # [DSEM, N]
    spk_new = spk[perm_global]
    emo_new = emo[perm_global]

    oh16 = np.zeros((16, N), BF16)
    oh16[spk_new, np.arange(N)] = 1.0
    oh16[NUM_SPK + emo_new, np.arange(N)] = 1.0

    # adjacency in permuted coords
    s_new = old2new[g_e, s_l]
    t_new = old2new[g_e, t_l]
    A = np.zeros((B, P, P), np.float32)
    np.add.at(A, (g_e, t_new, s_new), 1.0)
    # layer-1 AT tiles: block-diag, 4 graphs per 128x128 tile
    G = B // 4
    atb = np.zeros((G, 128, 128), np.float32)
    Ar = A.reshape(G, 4, P, P)
    for i in range(4):
        atb[:, 32 * i:32 * i + 32, 32 * i:32 * i + 32] = Ar[:, i].transpose(0, 2, 1)
    atb = atb.astype(BF16)
    # layer-2 AT tiles: [tile, 128 src(full layout), 4*C2 tgt(prefix C2)]
    atb2 = np.zeros((G, 128, 4 * C2), np.float32)
    for i in range(4):
        atb2[:, 32 * i:32 * i + 32, C2 * i:C2 * i + C2] = (
            Ar[:, i][:, :C2, :].transpose(0, 2, 1)
        )
    atb2 = atb2.astype(BF16)
    # layer-3 AT tiles: [tile, 128 src(packed C2), 2*gp3 tgt(slots 0,1)]
    gp3 = 128 // C2
    G3 = B // gp3
    atb3 = np.zeros((G3, 128, 2 * gp3), np.float32)
    Ar3 = A.reshape(G3, gp3, P, P)
    for i in range(gp3):
        atb3[:, C2 * i:C2 * i + C2, 2 * i:2 * i + 2] = (
            Ar3[:, i][:, :2, :C2].transpose(0, 2, 1)
        )
    atb3 = atb3.astype(BF16)
    # exactness check: every in-edge of slots {0,1} originates within prefix C2
    assert not A[:, :2, C2:].any()

    cmask = np.tile((c_idx == t_idx).astype(np.uint8)[None, :], (128, 1))

    ohd = np.zeros((P, B), BF16)
    ohd[dist, np.arange(B)] = 1.0

    extT = np.ascontiguousarray(ex.T.astype(BF16))

    embcat = np.concatenate(
        [np.asarray(inputs["spk_emb"], np.float32),
         np.asarray(inputs["emo_emb"], np.float32)], 0
    ).astype(BF16)
    rearr = lambda v: np.ascontiguousarray(
        np.asarray(v, np.float32).reshape(HC, 128).T
    )
    b16 = lambda k: np.asarray(inputs[k], np.float32).astype(BF16)

    shared = dict(
        embcat=embcat,
        wsem=b16("W_sem"),
        wself=b16("gnn_w_self"),
        wnbr=b16("gnn_w_nbr"),
        demb=b16("dist_emb"),
        wexpl=b16("W_expl"),
        bexpl=rearr(inputs["b_expl"]),
        wp1=b16("W_p1"),
        bp1=rearr(inputs["b_p1"]),
        wp2=rearr(np.asarray(inputs["W_p2"], np.float32)[:, 0]).astype(BF16),
        bp2=np.asarray(inputs["b_p2"], np.float32).reshape(1, 1),
    )

    in_maps = []
    for i in range(NCORES):
        gs = slice(i * BC, (i + 1) * BC)
        ns = slice(i * NCN, (i + 1) * NCN)
        ts = slice(i * (NCN // 128), (i + 1) * (NCN // 128))
        t3 = slice(i * (BC // gp3), (i + 1) * (BC // gp3))
        m = dict(shared)
        m["xt"] = np.ascontiguousarray(xtb[:, ns])
        m["oh16"] = np.ascontiguousarray(oh16[:, ns])
        m["atb"] = np.ascontiguousarray(atb[ts])
        m["atb2"] = np.ascontiguousarray(atb2[ts])
        m["atb3"] = np.ascontiguousarray(atb3[t3])
        m["cmask"] = np.ascontiguousarray(cmask[:, gs])
        m["ohd"] = np.ascontiguousarray(ohd[:, gs])
        m["ext"] = np.ascontiguousarray(extT[:, gs])
        in_maps.append(m)
    return in_maps, C2


def kernel(**inputs):
    in_maps, C2 = _host_prep(inputs)
    if C2 not in _cache:
        _cache[C2] = _build_program(C2)
    from concourse.bass_utils import run_bass_kernel_spmd

    res = run_bass_kernel_spmd(_cache[C2], in_maps, list(range(NCORES)))
    out = np.concatenate(
        [res.results[i]["out"].reshape(BC) for i in range(NCORES)]
    )
    return out.astype(np.float32)


# revision 9
# speedup vs baseline: 1.4584x; 1.0187x over previous
"""Trainium2 Bass kernel for nn_CrossTowerCausalModel.

Data-parallel over graphs: each of the 8 NeuronCores handles 128 graphs
(128*32 = 4096 nodes, 128*64 = 8192 edges). Weights/embeddings replicated.

Device activation layout is "transposed" (layout B): hT[feature, node] with
the 768 feature dim split into 6 chunks of 128 partitions. Weight matrices
[in, out] then serve directly as matmul lhsT (stationary) operands.

Receptive-field restriction: the GNN output h_graph is only read at 2 nodes
per graph (c, t). Host permutes each graph's 32 node slots so that
  slot 0 = c, slot 1 = t (filler if c == t),
  slots [0, C2) contain S2 = {c,t} U in({c,t})  (in-neighbors),
so layer 3 only computes slots {0,1} and layer 2 only the C2-slot prefix
(C2=16 when the per-graph |S2| max allows; else full 32). Layer 1 must stay
full (its targets feed layer-2 message sources anywhere). This is exact --
every value read downstream is identical to the full computation.

h is stored in bf16 (matmul input dtype), eliminating the per-layer f32->
bf16 copy; the per-layer psum accumulation and relu stay fp32.

Host-side prep (pure index logic + layout, no heavy math):
  * per-graph node permutation (above) -> final gathers h_c / h_t become
    stride-32 strided copies. (c == t graphs fixed up with copy_predicated.)
  * x passed pre-transposed (feature-major) bf16.
  * dense per-graph adjacency as block-diagonal 128x128 AT tiles (layer 1),
    [128, 4*C2] restricted tiles (layer 2), [128, 16] tiles (layer 3).
  * the quirky first-edge/dist logic of the reference (exact int math).
  * speaker/emotion one-hots (16 rows) fused into the input projection.
"""

import numpy as np
import ml_dtypes

B = 1024          # graphs
P = 32            # nodes per graph
N = B * P
H = 768
HC = H // 128     # 6 feature chunks
L = 3
DSEM = 1024
NUM_SPK, NUM_EMO = 9, 7
NCORES = 8
BC = B // NCORES          # graphs per core = 128
NCN = BC * P              # nodes per core = 4096
NT = 8                    # node tiles of 512 per core
GPT = 4                   # groups (of 128 nodes) per node tile

BF16 = ml_dtypes.bfloat16

_cache = {}


def _build_program(C2):
    from contextlib import ExitStack

    import concourse.bacc as bacc
    import concourse.mybir as mybir
    import concourse.tile as tile
    from concourse.masks import make_identity

    f32 = mybir.dt.float32
    bf16 = mybir.dt.bfloat16
    AF = mybir.ActivationFunctionType

    NC2 = BC * C2             # layer-2 packed cols per core
    NT2 = NC2 // 512          # layer-2 node tiles of 512
    RT3 = NC2 // 128          # layer-3 source row tiles
    GP3 = 128 // C2           # graphs per layer-3 source row tile

    nc = bacc.Bacc(
        "TRN2", target_bir_lowering=False, debug=False, num_devices=NCORES
    )

    dram = lambda name, shape, dt: nc.dram_tensor(
        name, shape, dt, kind="ExternalInput"
    ).ap()

    xt = dram("xt", [DSEM, NCN], bf16)
    oh16 = dram("oh16", [16, NCN], bf16)
    embcat = dram("embcat", [16, H], bf16)
    wsem = dram("wsem", [DSEM, H], bf16)
    wself = dram("wself", [L, H, H], bf16)
    wnbr = dram("wnbr", [L, H, H], bf16)
    atb = dram("atb", [NCN // 128, 128, 128], bf16)
    atb2 = dram("atb2", [NCN // 128, 128, 4 * C2], bf16)
    atb3 = dram("atb3", [RT3, 128, 2 * GP3], bf16)
    cmask = dram("cmask", [128, BC], mybir.dt.uint8)
    ohd = dram("ohd", [P, BC], bf16)
    demb = dram("demb", [P, H], bf16)
    wexpl = dram("wexpl", [H, H], bf16)
    bexpl = dram("bexpl", [128, HC], f32)
    ext = dram("ext", [H, BC], bf16)
    wp1 = dram("wp1", [HC, 128, 36 * 128], bf16)
    bp1 = dram("bp1", [128, HC], f32)
    wp2 = dram("wp2", [128, HC], bf16)
    bp2 = dram("bp2", [1, 1], f32)
    out_ap = nc.dram_tensor("out", [1, BC], f32, kind="ExternalOutput").ap()

    # [C*128, J] dram AP -> [128, C, J] (partition-major chunked view)
    def chunked(ap, J):
        return ap.rearrange("(c p) j -> c p j", p=128).transpose([1, 0, 2])

    # SBUF tile [128, C*J] -> [128, C, J]
    def sb3(t, J):
        return t[:].rearrange("p (c j) -> p c j", j=J)

    with tile.TileContext(nc) as tc, ExitStack() as ctx:
        erpool = ctx.enter_context(tc.tile_pool(name="er", bufs=1))
        cpool = ctx.enter_context(tc.tile_pool(name="const", bufs=1))

        # pools close in LIFO order: hA (after layer 1), then hB (after
        # layer 2); h2 lives until the end.
        h2pool = ctx.enter_context(tc.tile_pool(name="h2", bufs=1))
        hsB = ctx.enter_context(ExitStack())     # closes after layer 2
        hBpool = hsB.enter_context(tc.tile_pool(name="hB", bufs=1))
        hsA = ctx.enter_context(ExitStack())     # closes after layer 1
        hApool = hsA.enter_context(tc.tile_pool(name="hA", bufs=1))

        ident = cpool.tile([128, 128], bf16)
        make_identity(nc, ident)
        cmask_t = cpool.tile([128, BC], mybir.dt.uint8)
        nc.sync.dma_start(cmask_t[:], cmask[:])

        # transposed activations, ping-pong: hA = h_text (layer-1 input),
        # hB = h1 (layer-2 input); each [jc][nt] tile is [128, 512] bf16
        hA = [
            [
                hApool.tile(
                    [128, 512], bf16, tag=f"hA_{jc}_{nt}", name=f"hA_{jc}_{nt}"
                )
                for nt in range(NT)
            ]
            for jc in range(HC)
        ]
        hB = [
            [
                hBpool.tile(
                    [128, 512], bf16, tag=f"hB_{jc}_{nt}", name=f"hB_{jc}_{nt}"
                )
                for nt in range(NT)
            ]
            for jc in range(HC)
        ]
        # layer-2 packed activations: h2T[jc][nt2] is [128, 512] bf16
        h2T = [
            [
                h2pool.tile(
                    [128, 512], bf16, tag=f"h2_{jc}_{nt2}", name=f"h2_{jc}_{nt2}"
                )
                for nt2 in range(NT2)
            ]
            for jc in range(HC)
        ]
        # edge_repr^T, 36 chunks of 128 rows: [h_graph_c, h_text_c, h_graph_t,
        # h_text_t, h_dist, z] each HC chunks wide
        erT = erpool.tile([128, 36 * 128], bf16)

        # ---------------- phase 1: text projection ----------------
        with ExitStack() as p1:
            xtpool = p1.enter_context(tc.tile_pool(name="xt", bufs=3))
            wsem_pool = p1.enter_context(tc.tile_pool(name="wsem", bufs=1))
            oh_pool = p1.enter_context(tc.tile_pool(name="oh16", bufs=3))
            ps_a = p1.enter_context(tc.tile_pool(name="ps_a", bufs=4, space="PSUM"))

            wsem_t = wsem_pool.tile([128, 8 * H], bf16)
            nc.sync.dma_start(sb3(wsem_t, H), chunked(wsem, H))
            emb_t = wsem_pool.tile([128, H], bf16)
            nc.sync.dma_start(emb_t[:16, :], embcat[:])
            for nt in range(NT):
                oh16_t = oh_pool.tile([128, 512], bf16)
                nc.sync.dma_start(oh16_t[:16, :], oh16[:, nt * 512:][:, :512])
                xt_t = xtpool.tile([128, 8 * 512], bf16)
                nc.sync.dma_start(
                    sb3(xt_t, 512), chunked(xt[:, nt * 512:][:, :512], 512)
                )
                for jc in range(HC):
                    acc = ps_a.tile([128, 512], f32)
                    for kc in range(8):
                        nc.tensor.matmul(
                            acc[:],
                            wsem_t[:, kc * H + jc * 128:][:, :128],
                            xt_t[:, kc * 512:][:, :512],
                            start=(kc == 0),
                            stop=False,
                        )
                    nc.tensor.matmul(
                        acc[:],
                        emb_t[:16, jc * 128:][:, :128],
                        oh16_t[:16, :],
                        start=False,
                        stop=True,
                    )
                    nc.scalar.activation(hA[jc][nt][:], acc[:], AF.Relu)

            # h_text gathers (chunks 6-11 = h_text_c, 18-23 = h_text_t)
            for jc in range(HC):
                for nt in range(NT):
                    src = hA[jc][nt].rearrange("p (b u) -> p b u", u=P)
                    nc.vector.tensor_copy(
                        erT[:, (6 + jc) * 128 + nt * 16:][:, :16],
                        src[:, :, 0],
                    )
                    nc.vector.tensor_copy(
                        erT[:, (18 + jc) * 128 + nt * 16:][:, :16],
                        src[:, :, 1],
                    )
                nc.vector.copy_predicated(
                    erT[:, (18 + jc) * 128:][:, :BC],
                    cmask_t[:],
                    erT[:, (6 + jc) * 128:][:, :BC],
                )

        # ---------------- phase 2: GNN layer 1 (full) ----------------
        with ExitStack() as p2:
            wpool = p2.enter_context(tc.tile_pool(name="w", bufs=2))
            a1pool = p2.enter_context(tc.tile_pool(name="a1", bufs=1))
            hapool = p2.enter_context(tc.tile_pool(name="ha", bufs=3))
            msgpool = p2.enter_context(tc.tile_pool(name="msg", bufs=2))
            tmppool = p2.enter_context(tc.tile_pool(name="tmp", bufs=3))
            ps_t2 = p2.enter_context(tc.tile_pool(name="ps_t2", bufs=2, space="PSUM"))
            ps_m = p2.enter_context(tc.tile_pool(name="ps_m", bufs=2, space="PSUM"))
            ps_a2 = p2.enter_context(tc.tile_pool(name="ps_a2", bufs=3, space="PSUM"))

            atb_t = a1pool.tile([128, (NCN // 128) * 128], bf16)
            nc.sync.dma_start(sb3(atb_t, 128), atb.transpose([1, 0, 2]))

            ws_t = wpool.tile([128, HC * H], bf16, tag="ws")
            nc.sync.dma_start(sb3(ws_t, H), chunked(wself[0], H))
            wn_t = wpool.tile([128, HC * H], bf16, tag="wn")
            nc.sync.dma_start(sb3(wn_t, H), chunked(wnbr[0], H))
            for nt in range(NT):
                msg_t = msgpool.tile([128, HC * 512], bf16)
                for jc in range(HC):
                    pst = ps_t2.tile([128, 512], bf16)
                    for g4 in range(GPT):
                        nc.tensor.transpose(
                            pst[:, g4 * 128:][:, :128],
                            hA[jc][nt][:, g4 * 128:][:, :128],
                            ident[:],
                        )
                    ha = hapool.tile([128, 512], bf16)
                    nc.scalar.activation(ha[:], pst[:], AF.Copy)
                    psm = ps_m.tile([128, 512], f32)
                    for g4 in range(GPT):
                        nc.tensor.matmul(
                            psm[:, g4 * 128:][:, :128],
                            ha[:, g4 * 128:][:, :128],
                            atb_t[:, (nt * GPT + g4) * 128:][:, :128],
                            start=True,
                            stop=True,
                        )
                    nc.vector.tensor_copy(msg_t[:, jc * 512:][:, :512], psm[:])
                for jc in range(HC):
                    acc = ps_a2.tile([128, 512], f32)
                    for kc in range(HC):
                        nc.tensor.matmul(
                            acc[:],
                            ws_t[:, kc * H + jc * 128:][:, :128],
                            hA[kc][nt][:],
                            start=(kc == 0),
                            stop=False,
                        )
                    for kc in range(HC):
                        nc.tensor.matmul(
                            acc[:],
                            wn_t[:, kc * H + jc * 128:][:, :128],
                            msg_t[:, kc * 512:][:, :512],
                            start=False,
                            stop=(kc == HC - 1),
                        )
                    tmp = tmppool.tile([128, 512], f32)
                    nc.scalar.activation(tmp[:], acc[:], AF.Relu)
                    nc.vector.tensor_add(
                        out=hB[jc][nt][:], in0=tmp[:], in1=hA[jc][nt][:]
                    )

        # hA (h_text) no longer needed
        hsA.close()

        # ---------------- phase 3: GNN layer 2 (prefix C2) ----------------
        with ExitStack() as p3:
            wpool = p3.enter_context(tc.tile_pool(name="w2", bufs=2))
            a2pool = p3.enter_context(tc.tile_pool(name="a2", bufs=1))
            hapool = p3.enter_context(tc.tile_pool(name="ha2", bufs=3))
            hppool = p3.enter_context(tc.tile_pool(name="hp", bufs=2))
            msgpool = p3.enter_context(tc.tile_pool(name="msg2", bufs=2))
            tmppool = p3.enter_context(tc.tile_pool(name="tmp2", bufs=3))
            ps_t2 = p3.enter_context(tc.tile_pool(name="ps_t3", bufs=2, space="PSUM"))
            ps_m = p3.enter_context(tc.tile_pool(name="ps_m3", bufs=2, space="PSUM"))
            ps_a2 = p3.enter_context(tc.tile_pool(name="ps_a3", bufs=3, space="PSUM"))

            atb2_t = a2pool.tile([128, (NCN // 128) * 4 * C2], bf16)
            nc.sync.dma_start(sb3(atb2_t, 4 * C2), atb2.transpose([1, 0, 2]))

            zpool = p3.enter_context(tc.tile_pool(name="z", bufs=1))
            ps_z = p3.enter_context(tc.tile_pool(name="ps_z", bufs=1, space="PSUM"))

            ws_t = wpool.tile([128, HC * H], bf16, tag="ws2")
            nc.sync.dma_start(sb3(ws_t, H), chunked(wself[1], H))
            wn_t = wpool.tile([128, HC * H], bf16, tag="wn2")
            nc.sync.dma_start(sb3(wn_t, H), chunked(wnbr[1], H))

            ohd_t = zpool.tile([128, BC], bf16)
            nc.sync.dma_start(ohd_t[:P, :], ohd[:])
            demb_t = zpool.tile([128, H], bf16)
            nc.sync.dma_start(demb_t[:P, :], demb[:])
            bexpl_t = zpool.tile([128, HC], f32)
            nc.sync.dma_start(bexpl_t[:], bexpl[:])
            ext_t = zpool.tile([128, HC * BC], bf16)
            nc.sync.dma_start(sb3(ext_t, BC), chunked(ext, BC))
            wexpl_t = zpool.tile([128, HC * H], bf16)
            nc.sync.dma_start(sb3(wexpl_t, H), chunked(wexpl, H))

            # h_dist (erT chunks 24-29) and z_teacher (30-35): independent of
            # the GNN; interleave 3 psum-groups per nt2 so drains hide under
            # the layer-2 matmul streams.
            def emit_zdist(zi):
                jc = zi % HC
                if zi < HC:
                    psd = ps_z.tile([128, BC], f32, tag="zz")
                    nc.tensor.matmul(
                        psd[:],
                        demb_t[:P, jc * 128:][:, :128],
                        ohd_t[:P, :],
                        start=True,
                        stop=True,
                    )
                    nc.scalar.activation(
                        erT[:, (24 + jc) * 128:][:, :BC], psd[:], AF.Copy
                    )
                else:
                    psz = ps_z.tile([128, BC], f32, tag="zz")
                    for kc in range(HC):
                        nc.tensor.matmul(
                            psz[:],
                            wexpl_t[:, kc * H + jc * 128:][:, :128],
                            ext_t[:, kc * BC:][:, :BC],
                            start=(kc == 0),
                            stop=(kc == HC - 1),
                        )
                    nc.scalar.activation(
                        erT[:, (30 + jc) * 128:][:, :BC],
                        psz[:],
                        AF.Relu,
                        bias=bexpl_t[:, jc:jc + 1],
                    )
            W2 = 4 * C2  # target cols produced per source row-tile
            SP2 = 512 // (16 * C2)  # source node-tiles feeding one nt2 tile
            for nt2 in range(NT2):
                # message phase over the two full-layout source tiles
                msg_t = msgpool.tile([128, HC * 512], bf16)
                for jc in range(HC):
                    psm = ps_m.tile([128, 512], f32)
                    for half in range(SP2):
                        nt = nt2 * SP2 + half
                        pst = ps_t2.tile([128, 512], bf16)
                        for g4 in range(GPT):
                            nc.tensor.transpose(
                                pst[:, g4 * 128:][:, :128],
                                hB[jc][nt][:, g4 * 128:][:, :128],
                                ident[:],
                            )
                        ha = hapool.tile([128, 512], bf16)
                        nc.scalar.activation(ha[:], pst[:], AF.Copy)
                        for g4 in range(GPT):
                            nc.tensor.matmul(
                                psm[:, (half * GPT + g4) * W2:][:, :W2],
                                ha[:, g4 * 128:][:, :128],
                                atb2_t[:, (nt * GPT + g4) * W2:][:, :W2],
                                start=True,
                                stop=True,
                            )
                    nc.vector.tensor_copy(msg_t[:, jc * 512:][:, :512], psm[:])
                # gather h1 at the C2-prefix of each graph (residual + self rhs)
                hp_t = hppool.tile([128, HC * 512], bf16)
                for jc in range(HC):
                    for half in range(SP2):
                        nt = nt2 * SP2 + half
                        src = hB[jc][nt].rearrange("p (b u) -> p b u", u=P)
                        dst = hp_t[
                            :, jc * 512 + half * (512 // SP2):
                        ][:, :512 // SP2].rearrange("p (b u) -> p b u", u=C2)
                        nc.vector.tensor_copy(dst, src[:, :, :C2])
                for jc in range(HC):
                    acc = ps_a2.tile([128, 512], f32)
                    for kc in range(HC):
                        nc.tensor.matmul(
                            acc[:],
                            ws_t[:, kc * H + jc * 128:][:, :128],
                            hp_t[:, kc * 512:][:, :512],
                            start=(kc == 0),
                            stop=False,
                        )
                    for kc in range(HC):
                        nc.tensor.matmul(
                            acc[:],
                            wn_t[:, kc * H + jc * 128:][:, :128],
                            msg_t[:, kc * 512:][:, :512],
                            start=False,
                            stop=(kc == HC - 1),
                        )
                    tmp = tmppool.tile([128, 512], f32)
                    nc.scalar.activation(tmp[:], acc[:], AF.Relu)
                    nc.vector.tensor_add(
                        out=h2T[jc][nt2][:],
                        in0=tmp[:],
                        in1=hp_t[:, jc * 512:][:, :512],
                    )
                for zi in range(
                    nt2 * 12 // NT2, (nt2 + 1) * 12 // NT2
                ):
                    emit_zdist(zi)

        # hB (h1) no longer needed
        hsB.close()

        # ---------------- phase 4: GNN layer 3 (slots 0,1) + predictor ----
        with ExitStack() as p4:
            ppool = p4.enter_context(tc.tile_pool(name="pred", bufs=1))
            w3pool = p4.enter_context(tc.tile_pool(name="w3", bufs=1))
            a3pool = p4.enter_context(tc.tile_pool(name="a3", bufs=1))
            hapool = p4.enter_context(tc.tile_pool(name="ha3", bufs=3))
            tmppool = p4.enter_context(tc.tile_pool(name="tmp3", bufs=2))
            ps_t2 = p4.enter_context(tc.tile_pool(name="ps_t4", bufs=2, space="PSUM"))
            ps_m = p4.enter_context(tc.tile_pool(name="ps_m4", bufs=1, space="PSUM"))
            ps_c3 = p4.enter_context(tc.tile_pool(name="ps_c3", bufs=2, space="PSUM"))
            ps_p = p4.enter_context(tc.tile_pool(name="ps_p", bufs=3, space="PSUM"))

            # predictor weight prefetch (DMA overlaps layer-3 compute)
            wp1_t = []
            for jc in range(HC):
                w1s = ppool.tile(
                    [128, 36 * 128], bf16, tag=f"wp1_{jc}", name=f"wp1_{jc}"
                )
                nc.sync.dma_start(w1s[:], wp1[jc])
                wp1_t.append(w1s)
            bp1_t = ppool.tile([128, HC], f32)
            nc.sync.dma_start(bp1_t[:], bp1[:])
            wp2_t = ppool.tile([128, HC], bf16)
            nc.sync.dma_start(wp2_t[:], wp2[:])
            bp2_t = ppool.tile([1, 1], f32)
            nc.sync.dma_start(bp2_t[:], bp2[:])
            atb3_t = a3pool.tile([128, RT3 * 2 * GP3], bf16)
            nc.sync.dma_start(sb3(atb3_t, 2 * GP3), atb3.transpose([1, 0, 2]))
            ws3_t = w3pool.tile([128, HC * H], bf16, tag="ws3")
            nc.sync.dma_start(sb3(ws3_t, H), chunked(wself[2], H))
            wn3_t = w3pool.tile([128, HC * H], bf16, tag="wn3")
            nc.sync.dma_start(sb3(wn3_t, H), chunked(wnbr[2], H))

            # --- layer 3 message + gather at slots {0,1} ---
            W3 = 2 * GP3  # target cols per source row tile
            msg3_t = ppool.tile([128, HC * 2 * BC], bf16)
            h2p_t = ppool.tile([128, HC * 2 * BC], bf16)
            for jc in range(HC):
                psm = ps_m.tile([128, 2 * BC], f32)
                for nt2 in range(NT2):
                    pst = ps_t2.tile([128, 512], bf16)
                    for g4 in range(GPT):
                        nc.tensor.transpose(
                            pst[:, g4 * 128:][:, :128],
                            h2T[jc][nt2][:, g4 * 128:][:, :128],
                            ident[:],
                        )
                    ha = hapool.tile([128, 512], bf16)
                    nc.scalar.activation(ha[:], pst[:], AF.Copy)
                    for g4 in range(GPT):
                        rt = nt2 * GPT + g4
                        nc.tensor.matmul(
                            psm[:, rt * W3:][:, :W3],
                            ha[:, g4 * 128:][:, :128],
                            atb3_t[:, rt * W3:][:, :W3],
                            start=True,
                            stop=True,
                        )
                nc.vector.tensor_copy(msg3_t[:, jc * 2 * BC:][:, :2 * BC], psm[:])
                PW3 = 2 * BC // NT2
                for nt2 in range(NT2):
                    src = h2T[jc][nt2].rearrange("p (b u) -> p b u", u=C2)
                    dst = h2p_t[
                        :, jc * 2 * BC + nt2 * PW3:
                    ][:, :PW3].rearrange("p (b u) -> p b u", u=2)
                    nc.vector.tensor_copy(dst, src[:, :, :2])

            # --- layer 3 W-matmuls -> h3 -> erT chunks 0-5 (c), 12-17 (t) ---
            for jc in range(HC):
                acc = ps_c3.tile([128, 2 * BC], f32)
                for kc in range(HC):
                    nc.tensor.matmul(
                        acc[:],
                        ws3_t[:, kc * H + jc * 128:][:, :128],
                        h2p_t[:, kc * 2 * BC:][:, :2 * BC],
                        start=(kc == 0),
                        stop=False,
                    )
                for kc in range(HC):
                    nc.tensor.matmul(
                        acc[:],
                        wn3_t[:, kc * H + jc * 128:][:, :128],
                        msg3_t[:, kc * 2 * BC:][:, :2 * BC],
                        start=False,
                        stop=(kc == HC - 1),
                    )
                tmp = tmppool.tile([128, 2 * BC], f32)
                nc.scalar.activation(tmp[:], acc[:], AF.Relu)
                h3 = tmppool.tile([128, 2 * BC], bf16)
                nc.vector.tensor_add(
                    out=h3[:], in0=tmp[:], in1=h2p_t[:, jc * 2 * BC:][:, :2 * BC]
                )
                h3v = h3.rearrange("p (b u) -> p b u", u=2)
                nc.vector.tensor_copy(erT[:, (0 + jc) * 128:][:, :BC], h3v[:, :, 0])
                nc.vector.tensor_copy(erT[:, (12 + jc) * 128:][:, :BC], h3v[:, :, 1])
                nc.vector.copy_predicated(
                    erT[:, (12 + jc) * 128:][:, :BC],
                    cmask_t[:],
                    erT[:, (0 + jc) * 128:][:, :BC],
                )

            hid_t = ppool.tile([128, HC * BC], bf16)
            for jc in range(HC):
                psp = ps_p.tile([128, BC], f32, tag="pp")
                for kc in range(36):
                    nc.tensor.matmul(
                        psp[:],
                        wp1_t[jc][:, kc * 128:][:, :128],
                        erT[:, kc * 128:][:, :128],
                        start=(kc == 0),
                        stop=(kc == 35),
                    )
                nc.scalar.activation(
                    hid_t[:, jc * BC:][:, :BC],
                    psp[:],
                    AF.Relu,
                    bias=bp1_t[:, jc:jc + 1],
                )

            psl = ps_p.tile([128, BC], f32, tag="pp")
            for jc in range(HC):
                nc.tensor.matmul(
                    psl[:1, :],
                    wp2_t[:, jc:jc + 1],
                    hid_t[:, jc * BC:][:, :BC],
                    start=(jc == 0),
                    stop=(jc == HC - 1),
                )
            logit_t = ppool.tile([128, BC], f32)
            nc.vector.tensor_scalar_add(
                out=logit_t[:1, :], in0=psl[:1, :], scalar1=bp2_t[:1, :1]
            )
            nc.sync.dma_start(out_ap[:], logit_t[:1, :])

    nc.compile()
    return nc


def _host_prep(inputs):
    x = np.asarray(inputs["x"], np.float32)
    spk = np.asarray(inputs["speaker_ids"], np.int64)
    emo = np.asarray(inputs["emotion_ids"], np.int64)
    ei = np.asarray(inputs["edge_index"], np.int64)
    tni = np.asarray(inputs["target_node_indices"], np.int64)
    ex = np.asarray(inputs["expl_space_vec"], np.float32)

    E = ei.shape[1]
    edge_src, edge_tgt = ei[0], ei[1]
    c_idx, t_idx = tni[:, 0], tni[:, 1]

    # reference first-edge/dist logic (exact)
    fe = np.full(N, E, np.int64)
    np.minimum.at(fe, edge_src, np.arange(E, dtype=np.int64))

    def first_tgt(q):
        feq = fe[q]
        return np.where(feq < E, edge_tgt[np.minimum(feq, E - 1)], q)

    dist = np.clip(np.abs(first_tgt(c_idx) - first_tgt(t_idx)), 0, P - 1)

    # slot-1 node: t, or a filler distinct from c when c == t
    t_eff = np.where(c_idx == t_idx, (t_idx + 1) % P, t_idx)

    # per-graph in-neighbor sets of {c, t_eff} -> S2 (old coords)
    g_e = edge_src // P
    s_l, t_l = edge_src % P, edge_tgt % P
    innb = np.zeros((B, P, P), bool)
    innb[g_e, t_l, s_l] = True
    sel = np.zeros((B, P), bool)
    bidx = np.arange(B)
    sel[bidx, c_idx] = True
    sel[bidx, t_eff] = True
    S2 = sel.copy()
    S2 |= np.einsum("bts,bt->bs", innb.astype(np.int8), sel.astype(np.int8)) > 0
    s2_max = int(S2.sum(1).max())
    C2 = 16 if s2_max <= 16 else 32

    # per-graph permutation: slot 0 = c, slot 1 = t_eff, S2 within prefix C2
    prio = np.full((B, P), 4, np.int64)
    prio[S2] = 2
    prio[bidx, t_eff] = 1
    prio[bidx, c_idx] = 0
    new2old = np.argsort(prio, axis=1, kind="stable")
    old2new = np.argsort(new2old, axis=1)
    perm_global = (np.arange(B)[:, None] * P + new2old).reshape(-1)

    xtb = np.ascontiguousarray(x[perm_global].T.astype(BF16))  # [DSEM, N]
    spk_new = spk[perm_global]
    emo_new = emo[perm_global]

    oh16 = np.zeros((16, N), BF16)
    oh16[spk_new, np.arange(N)] = 1.0
    oh16[NUM_SPK + emo_new, np.arange(N)] = 1.0

    # adjacency in permuted coords
    s_new = old2new[g_e, s_l]
    t_new = old2new[g_e, t_l]
    A = np.zeros((B, P, P), np.float32)
    np.add.at(A, (g_e, t_new, s_new), 1.0)
    # layer-1 AT tiles: block-diag, 4 graphs per 128x128 tile
    G = B // 4
    atb = np.zeros((G, 128, 128), np.float32)
    Ar = A.reshape(G, 4, P, P)
    for i in range(4):
        atb[:, 32 * i:32 * i + 32, 32 * i:32 * i + 32] = Ar[:, i].transpose(0, 2, 1)
    atb = atb.astype(BF16)
    # layer-2 AT tiles: [tile, 128 src(full layout), 4*C2 tgt(prefix C2)]
    atb2 = np.zeros((G, 128, 4 * C2), np.float32)
    for i in range(4):
        atb2[:, 32 * i:32 * i + 32, C2 * i:C2 * i + C2] = (
            Ar[:, i][:, :C2, :].transpose(0, 2, 1)
        )
    atb2 = atb2.astype(BF16)
    # layer-3 AT tiles: [tile, 128 src(packed C2), 2*gp3 tgt(slots 0,1)]
    gp3 = 128 // C2
    G3 = B // gp3
    atb3 = np.zeros((G3, 128, 2 * gp3), np.float32)
    Ar3 = A.reshape(G3, gp3, P, P)
    for i in range(gp3):
        atb3[:, C2 * i:C2 * i + C2, 2 * i:2 * i + 2] = (
            Ar3[:, i][:, :2, :C2].transpose(0, 2, 1)
        )
    atb3 = atb3.astype(BF16)
    # exactness check: every in-edge of slots {0,1} originates within prefix C2
    assert not A[:, :2, C2:].any()

    cmask = np.tile((c_idx == t_idx).astype(np.uint8)[None, :], (128, 1))

    ohd = np.zeros((P, B), BF16)
    ohd[dist, np.arange(B)] = 1.0

    extT = np.ascontiguousarray(ex.T.astype(BF16))

    embcat = np.concatenate(
        [np.asarray(inputs["spk_emb"], np.float32),
         np.asarray(inputs["emo_emb"], np.float32)], 0
    ).astype(BF16)
    rearr = lambda v: np.ascontiguousarray(
        np.asarray(v, np.float32).reshape(HC, 128).T
    )
    b16 = lambda k: np.asarray(inputs[k], np.float32).astype(BF16)

    shared = dict(
        embcat=embcat,
        wsem=b16("W_sem"),
        wself=b16("gnn_w_self"),
        wnbr=b16("gnn_w_nbr"),
        demb=b16("dist_emb"),
        wexpl=b16("W_expl"),
        bexpl=rearr(inputs["b_expl"]),
        wp1=np.ascontiguousarray(
            np.asarray(inputs["W_p1"], np.float32)
            .reshape(36, 128, HC, 128).transpose(2, 1, 0, 3)
            .reshape(HC, 128, 36 * 128)
        ).astype(BF16),
        bp1=rearr(inputs["b_p1"]),
        wp2=rearr(np.asarray(inputs["W_p2"], np.float32)[:, 0]).astype(BF16),
        bp2=np.asarray(inputs["b_p2"], np.float32).reshape(1, 1),
    )

    in_maps = []
    for i in range(NCORES):
        gs = slice(i * BC, (i + 1) * BC)
        ns = slice(i * NCN, (i + 1) * NCN)
        ts = slice(i * (NCN // 128), (i + 1) * (NCN // 128))
        t3 = slice(i * (BC // gp3), (i + 1) * (BC // gp3))
        m = dict(shared)
        m["xt"] = np.ascontiguousarray(xtb[:, ns])
        m["oh16"] = np.ascontiguousarray(oh16[:, ns])
        m["atb"] = np.ascontiguousarray(atb[ts])
        m["atb2"] = np.ascontiguousarray(atb2[ts])
        m["atb3"] = np.ascontiguousarray(atb3[t3])
        m["cmask"] = np.ascontiguousarray(cmask[:, gs])
        m["ohd"] = np.ascontiguousarray(ohd[:, gs])
        m["ext"] = np.ascontiguousarray(extT[:, gs])
        in_maps.append(m)
    return in_maps, C2


def kernel(**inputs):
    in_maps, C2 = _host_prep(inputs)
    if C2 not in _cache:
        _cache[C2] = _build_program(C2)
    from concourse.bass_utils import run_bass_kernel_spmd

    res = run_bass_kernel_spmd(_cache[C2], in_maps, list(range(NCORES)))
    out = np.concatenate(
        [res.results[i]["out"].reshape(BC) for i in range(NCORES)]
    )
    return out.astype(np.float32)
